# revision 36
# baseline (speedup 1.0000x reference)
"""Chunked (= full, non-causal) multi-head self-attention on 8 TRN2 NeuronCores.

Problem: B=2, S=2048, D=1024, H=16 heads (head_dim 64), torch-Linear-style
projections (y = x @ W.T + b), softmax attention, output projection.

Sharding: head-parallel. Core c owns heads {2c, 2c+1} = feature slice
[128c, 128c+128). Each core computes q/k/v for its slice from the full x
(replicated), runs attention for its 4 (batch, head) pairs, and produces a
partial output projection with its 128-row slice of Wo. Host sums the 8
partials and adds bo.

Layout: scores are computed transposed, ST[k, q] (keys on partitions), so the
softmax exp output PT feeds the P@V matmul directly (contraction over k on
partitions) with no on-chip transposes anywhere — x and the weights are
pre-chunked on the host so every DMA moves fat contiguous per-partition
blocks. The softmax denominator rides as row 64 of the PV output via a
ones-column appended to V (M=65); normalization packs both heads' rowsums
into a [2, NF] tile so the reciprocal is ONE Ln + ONE Exp on ACT, emitted
into ACT's natural wait-for-scores bubble at the iteration boundary.

Scheduling: a single software-pipelined stream over all 64 (batch, qc, kp)
score/PV pair-slots. Attention starts as soon as sblock 0's q/k/v exist
(~5us); every remaining projection piece and output-projection tile is
metered into the stream as PE filler with per-piece deadlines, so the PE
never idles (TRN2's HAM clock-gate halves the PE clock when it does) and the
ACT exp stream never outruns it. A few y-tiles are held back to cover the
final normalization chain's latency at the drain.

Precision: everything bf16 in, fp32 accumulate/out (~5e-3 rel err).
"""

import sys

if "/opt/trn_rl_repo" not in sys.path:
    sys.path.insert(0, "/opt/trn_rl_repo")

import numpy as np

import concourse.bacc as bacc
import concourse.mybir as mybir
import concourse.tile as tile
from concourse import bass_utils

# Route Exp to the activation-table set that also holds Ln, so the softmax
# exps and the reciprocal-via-exp(-ln(x)) trick share one table (the default
# per-function choice would ping-pong table loads at ~2.7us each).
_orig_get_activation_tables = bacc.get_activation_tables


def _patched_get_activation_tables(arch):
    out = {}
    for name, funcs in dict(_orig_get_activation_tables(arch)).items():
        if name != "natural_log_exp_and_others":
            funcs = {f for f in funcs if f != mybir.ActivationFunctionType.Exp}
        out[name] = funcs
    return out


bacc.get_activation_tables = _patched_get_activation_tables

B, S, D, H = 2, 2048, 1024, 16
HD = D // H          # 64
NCORES = 8
ES = D // NCORES     # 128 features (= 2 heads) per core
BS = B * S           # 4096 rows total

P = 128              # partitions
NF = 512             # matmul free-dim tile
N_SB = BS // NF      # 8 s-blocks of 512
N_DC = D // P        # 8 contraction chunks of 128
N_KB = S // P        # 16 key blocks of 128 per batch
N_KP = N_KB // 2     # 8 key-block PAIRS per batch
N_QC = S // NF       # 4 query chunks of 512 per batch
N_CH = BS // P       # 32 global 128-row chunks

F32 = mybir.dt.float32
BF16 = mybir.dt.bfloat16

DT_QK = BF16         # x/Wq/Wk inputs for q,k projections + score matmuls
DT_V = BF16          # x/Wv inputs for v projection
DT_ATT = BF16        # attention weights (exp output) and V in the P@V matmul
DT_OUT = BF16        # output projection inputs (OT, Wo)

_cache = {}
last_results = None          # test.py reads exec_time_ns off this


def _np_dt(dt):
    import ml_dtypes

    return np.dtype(ml_dtypes.bfloat16) if dt == mybir.dt.bfloat16 else np.dtype(np.float32)


def _build():
    nc = bacc.Bacc("TRN2", target_bir_lowering=False, debug=False)

    # Host-pre-chunked layouts: per-partition-contiguous so every DMA uses
    # 128 fat descriptors instead of 1024 thin ones.
    xs_d = nc.dram_tensor("xs", [P, N_SB * N_DC * NF], DT_QK, kind="ExternalInput")
    wqc_d = nc.dram_tensor("wqc", [P, N_DC * ES], DT_QK, kind="ExternalInput")
    wkc_d = nc.dram_tensor("wkc", [P, N_DC * ES], DT_QK, kind="ExternalInput")
    wvc_d = nc.dram_tensor("wvc", [P, N_DC * ES], DT_V, kind="ExternalInput")
    bq_d = nc.dram_tensor("bq", [ES, 1], F32, kind="ExternalInput")
    bk_d = nc.dram_tensor("bk", [ES, 1], F32, kind="ExternalInput")
    bv_d = nc.dram_tensor("bv", [1, ES], F32, kind="ExternalInput")
    woT_d = nc.dram_tensor("woT", [ES, D], DT_OUT, kind="ExternalInput")
    y_d = nc.dram_tensor("y", [BS, D], F32, kind="ExternalOutput")

    xs_r = xs_d.ap().rearrange("p (b a s) -> p b a s", b=N_SB, a=N_DC)
    wq_r = wqc_d.ap().rearrange("p (a e) -> p a e", a=N_DC)
    wk_r = wkc_d.ap().rearrange("p (a e) -> p a e", a=N_DC)
    wv_r = wvc_d.ap().rearrange("p (a e) -> p a e", a=N_DC)

    with tile.TileContext(nc) as tc:
        with tc.tile_pool(name="const", bufs=1) as cpool, \
             tc.tile_pool(name="xt", bufs=4) as xt_pool, \
             tc.tile_pool(name="qkv", bufs=1) as qkv_pool, \
             tc.tile_pool(name="pt", bufs=14) as pt_pool, \
             tc.tile_pool(name="ysb", bufs=6) as y_pool, \
             tc.tile_pool(name="ps", bufs=1, space="PSUM") as ps:

            # ---- weights / constants ------------------------------------
            # The critical-path DMAs (strip0 for k0's moving data, then
            # wk/wq) go FIRST on the SP queue in need-order; everything else
            # rides the ACT queue so its DGE setup overlaps SP's.
            strips = {}

            def emit_strip_dma(sb):
                strip = xt_pool.tile([P, N_DC, NF], DT_QK, tag="strip", name=f"strip{sb}")
                nc.sync.dma_start(strip[:], xs_r[:, sb])
                strips[sb] = strip

            wk_sb = cpool.tile([P, N_DC, ES], DT_QK)
            wq_sb = cpool.tile([P, N_DC, ES], DT_QK)
            nc.sync.dma_start(wk_sb[:], wk_r[:, :, :])
            nc.sync.dma_start(wq_sb[:], wq_r[:, :, :])
            emit_strip_dma(0)
            emit_strip_dma(1)

            bq_sb = cpool.tile([ES, 1], F32)
            bk_sb = cpool.tile([ES, 1], F32)
            bv_row = cpool.tile([1, ES], F32)
            nc.scalar.dma_start(bq_sb[:], bq_d[:])
            nc.scalar.dma_start(bk_sb[:], bk_d[:])
            nc.scalar.dma_start(bv_row[:], bv_d[:])
            wv_sb = cpool.tile([P, N_DC, ES], DT_V)
            nc.scalar.dma_start(wv_sb[:], wv_r[:, :, :])
            wo_sb = cpool.tile([ES, D], DT_OUT)
            nc.scalar.dma_start(wo_sb[:], woT_d[:])

            ones_row = cpool.tile([1, ES], F32)
            nc.vector.memset(ones_row[:], 1.0)

            # ---- persistent activations ---------------------------------
            qT_sb = qkv_pool.tile([P, BS], DT_QK)     # [feat 128, s 4096]
            kT_sb = qkv_pool.tile([P, BS], DT_QK)
            vA_sb = qkv_pool.tile([P, N_CH, HD + 1], DT_ATT)  # head A V + ones col
            vB_sb = qkv_pool.tile([P, N_CH, HD + 1], DT_ATT)
            oT_sb = qkv_pool.tile([P, BS], DT_OUT)    # normalized attn out, [feat, s]
            nc.vector.memset(vA_sb[:, :, HD : HD + 1], 1.0)
            nc.vector.memset(vB_sb[:, :, HD : HD + 1], 1.0)
            # pre-touch the packed-rowsum ring (heads live at partitions 0 and
            # 64; the ACT Ln reads the whole [65, NF] block, so the unused
            # partitions must have been written once)
            for _ in range(2):
                zp_init = pt_pool.tile([HD + 1, NF], F32, tag="zp", bufs=2)
                nc.vector.memset(zp_init[:], 1.0)

            # ---- emission helpers ---------------------------------------
            def emit_qk_piece(sb, which):
                s0 = sb * NF
                strip = strips[sb]
                w_sb, bias, dst = ((wq_sb, bq_sb, qT_sb) if which == "q"
                                   else (wk_sb, bk_sb, kT_sb))
                p_ps = ps.tile([P, NF], F32, tag="misc", bufs=2, name=f"{which}{sb}_ps")
                for j in range(N_DC):
                    nc.tensor.matmul(p_ps[:], w_sb[:, j], strip[:, j],
                                     start=(j == 0), stop=(j == N_DC - 1))
                nc.vector.tensor_scalar_add(dst[:, s0 : s0 + NF], p_ps[:], bias[:])

            def emit_v_piece(sb, ss):
                strip = strips[sb]
                ch = sb * (NF // P) + ss
                v_ps = ps.tile([P, ES], F32, tag="misc", bufs=2, name=f"v{ch}_ps")
                for j in range(N_DC):
                    nc.tensor.matmul(v_ps[:], strip[:, j, ss * P : (ss + 1) * P],
                                     wv_sb[:, j],
                                     start=(j == 0), stop=(j == N_DC - 1))
                nc.vector.tensor_add(vA_sb[:, ch, 0:HD], v_ps[:, 0:HD], bv_bc[:, 0:HD])
                nc.vector.tensor_add(vB_sb[:, ch, 0:HD], v_ps[:, HD:ES], bv_bc[:, HD:ES])

            # y output: both 512-wide halves of one 128-row block in one go —
            # one SBUF tile, ONE dma (halves the SP queue's per-dma cost).
            # In-loop the two matmuls ride the misc PSUM ring with DVE
            # eviction; at the drain they use the freed st2 ring with the
            # evictions split across DVE and ACT so the PE never waits.
            drain_dma = [0]

            def emit_ypair(s0, drain=False):
                y2_sb = y_pool.tile([P, 2, NF], F32, tag="y")
                if drain:
                    y_ps = ps.tile([P, 2, NF], F32, tag="st2", bufs=2)
                    for ec in range(2):
                        nc.tensor.matmul(y_ps[:, ec], oT_sb[:, s0 : s0 + P],
                                         wo_sb[:, ec * NF : (ec + 1) * NF],
                                         start=True, stop=True)
                    nc.vector.tensor_copy(y2_sb[:, 0], y_ps[:, 0])
                    nc.scalar.activation(y2_sb[:, 1], y_ps[:, 1],
                                         mybir.ActivationFunctionType.Copy)
                    eng = nc.sync if drain_dma[0] % 2 == 0 else nc.scalar
                    drain_dma[0] += 1
                    eng.dma_start(y_d[s0 : s0 + P, :], y2_sb.rearrange("p a b -> p (a b)"))
                else:
                    for ec in range(2):
                        y_ps = ps.tile([P, NF], F32, tag="misc", bufs=2)
                        nc.tensor.matmul(y_ps[:], oT_sb[:, s0 : s0 + P],
                                         wo_sb[:, ec * NF : (ec + 1) * NF],
                                         start=True, stop=True)
                        nc.vector.tensor_copy(y2_sb[:, ec], y_ps[:])
                    nc.sync.dma_start(y_d[s0 : s0 + P, :], y2_sb.rearrange("p a b -> p (a b)"))

            # ---- filler plan (deadline = pair index it must be done by) --
            # pair g = b*32 + qc*8 + kp. k(sb)/v(sb,ss) feed the kp sweep of
            # EVERY qc iteration of batch sb//4, so they're due at qc=0's
            # consuming pair; q(sb) is due when iteration (b, qc=sb%4) starts.
            filler = []            # (deadline, kind, cost_cycles, args)

            def dl_k(sb):
                return (sb // 4) * 32 + (sb % 4) * 2

            def dl_v(sb, ss):
                return (sb // 4) * 32 + ((sb % 4) * 4 + ss) // 2

            def dl_q(sb):
                return (sb // 4) * 32 + (sb % 4) * 8

            for sb in (1, 2, 3):
                filler.append((max(0, dl_k(sb) - 1), "dma", 0, (sb + 1,)))
                filler.append((dl_k(sb), "k", 4096, (sb,)))
                for ss in range(4):
                    filler.append((dl_v(sb, ss), "v", 1024, (sb, ss)))
            filler.append((8, "q", 4096, (1,)))
            filler.append((16, "q", 4096, (2,)))
            filler.append((24, "q", 4096, (3,)))
            for sb in (4, 5, 6, 7):
                filler.append((dl_k(sb) - 4, "dma", 0, (sb,)))
                filler.append((dl_k(sb), "k", 4096, (sb,)))
                if sb == 4:
                    filler.append((dl_q(sb), "q", 4096, (sb,)))
                for ss in range(4):
                    filler.append((dl_v(sb, ss), "v", 1024, (sb, ss)))
            for sb in (5, 6, 7):
                filler.append((dl_q(sb), "q", 4096, (sb,)))
            filler.sort(key=lambda e: e[0])

            bv_bc_done = [False]

            def emit_filler(ent):
                _, kind, _, args = ent
                if kind == "dma":
                    emit_strip_dma(*args)
                elif kind == "k":
                    emit_qk_piece(args[0], "k")
                elif kind == "q":
                    emit_qk_piece(args[0], "q")
                else:
                    if not bv_bc_done[0]:
                        emit_bv_bc()
                        bv_bc_done[0] = True
                    emit_v_piece(*args)

            # ---- startup -------------------------------------------------
            # Dummy rank-1 matmuls on the ones row warm the HAM clock gate
            # (~3.4us of sustained PE activity lifts the PE from 1.2 to
            # 2.4 GHz) while strip0/wk are still in flight, so the first
            # real projections run at full clock.
            warm = cpool.tile([1, NF], F32)
            nc.vector.memset(warm[:], 0.0)
            for _ in range(10):
                w_ps = ps.tile([P, NF], F32, tag="misc", bufs=2)
                nc.tensor.matmul(w_ps[:], ones_row[:], warm[:], start=True, stop=True)

            # k0/q0 first so the first scores (and thus ACT's exp stream)
            # start as early as possible; bv_bc and sblock 0's v pieces
            # follow as pair-0/1 filler.
            emit_qk_piece(0, "k")
            emit_qk_piece(0, "q")

            bv_bc = cpool.tile([P, ES], F32)

            def emit_bv_bc():
                # bv broadcast to all 128 partitions via rank-1 matmul
                bv_bc_ps = ps.tile([P, ES], F32, tag="misc", bufs=2)
                nc.tensor.matmul(bv_bc_ps[:], ones_row[:], bv_row[:],
                                 start=True, stop=True)
                nc.vector.tensor_copy(bv_bc[:], bv_bc_ps[:])

            for ss in range(4):
                filler.insert(ss, (0, "v", 1024, (0, ss)))

            inv_sqrt_hd = 1.0 / float(np.sqrt(HD))

            # ---- attention: one continuous software-pipelined stream -----
            n_iters = B * N_QC
            total_pairs = n_iters * N_KP
            o_tiles = {}
            ptq = {}
            pending = None           # iter finished, awaiting recip chain
            norm_state = None        # recip done, awaiting broadcast+apply
            bc_state = None
            y_avail = []             # y tiles whose apply has been emitted
            y_done = 0

            def emit_recip(oA_raw, oB_raw, zp, q0, it):
                # 1/rowsum via the single-op DVE Newton-Raphson reciprocal
                # (~18 bits — ample for softmax normalization; the rowsums
                # are benign positive values). Keeping the whole chain on
                # DVE->Pool->DVE leaves ACT's exp stream uninterrupted.
                zr = pt_pool.tile([HD + 1, NF], F32, tag="zr", bufs=2)
                nc.vector.reciprocal_approx_fast(zr[:], zp[:])
                return (oA_raw, oB_raw, zr, q0, it)

            def emit_bcast(oA_raw, oB_raw, zr, q0, it):
                # partition_broadcast always sources the tile's partition 0,
                # so head B's recip (at partition 64) is de-packed first. The
                # de-pack runs on DVE: the Pool engine must execute ONLY
                # partition_broadcast — any other op type forces a ~7us
                # GPSIMD ucode library swap (MODIFY_POOL_CONFIG).
                zrB = pt_pool.tile([1, NF], F32, tag="zrB", bufs=2)
                nc.vector.tensor_copy(zrB[:], zr[HD : HD + 1, :])
                bc2 = pt_pool.tile([HD, 2, NF], F32, tag="bc", bufs=3)
                nc.gpsimd.partition_broadcast(bc2[:, 0], zr[0:1, :])
                nc.gpsimd.partition_broadcast(bc2[:, 1], zrB[:])
                return (oA_raw, oB_raw, bc2, q0, it)

            def emit_apply(oA_raw, oB_raw, bc2, q0, it):
                for hidx, (o_raw, part) in enumerate(((oA_raw, 0), (oB_raw, HD))):
                    nc.vector.tensor_mul(
                        oT_sb[part : part + HD, q0 : q0 + NF],
                        o_raw[0:HD, :], bc2[:, hidx])
                for ss in range(NF // P):
                    y_avail.append(q0 + ss * P)

            for g in range(total_pairs + 1):
                kp = g % N_KP
                # iteration-boundary chain (pending is set at the kp==0 slot,
                # when the previous iteration's last PV lands), emitted at
                # slot start so the ACT ops land before this slot's exps and
                # the Pool/DVE ops run under the PE's score matmuls.
                if kp == 1 and pending is not None:
                    norm_state = emit_recip(*pending)
                    pending = None
                if kp == 2 and norm_state is not None:
                    bc_state = emit_bcast(*norm_state)
                    norm_state = None
                if kp == 3 and bc_state is not None:
                    emit_apply(*bc_state)
                    bc_state = None

                def sec_st():
                    if g >= total_pairs:
                        return
                    it = g // N_KP
                    b, qc = it // N_QC, it % N_QC
                    q0 = b * S + qc * NF
                    st2A = ps.tile([P, 2, NF], F32, tag="st2", bufs=2)
                    st2B = ps.tile([P, 2, NF], F32, tag="st2", bufs=2)
                    for half in range(2):
                        k0 = b * S + (kp * 2 + half) * P
                        nc.tensor.matmul(st2A[:, half], kT_sb[0:HD, k0 : k0 + P],
                                         qT_sb[0:HD, q0 : q0 + NF],
                                         start=True, stop=True)
                        nc.tensor.matmul(st2B[:, half], kT_sb[HD:P, k0 : k0 + P],
                                         qT_sb[HD:P, q0 : q0 + NF],
                                         start=True, stop=True)
                    pt2A = pt_pool.tile([P, 2, NF], DT_ATT, tag="pt", bufs=14)
                    pt2B = pt_pool.tile([P, 2, NF], DT_ATT, tag="pt", bufs=14)
                    nc.scalar.activation(pt2A[:], st2A[:],
                                         mybir.ActivationFunctionType.Exp,
                                         scale=inv_sqrt_hd)
                    nc.scalar.activation(pt2B[:], st2B[:],
                                         mybir.ActivationFunctionType.Exp,
                                         scale=inv_sqrt_hd)
                    ptq[g] = (pt2A, pt2B)

                def sec_pv():
                    nonlocal pending
                    if g < 1:
                        return
                    pg = g - 1
                    pit = pg // N_KP
                    pkp = pg % N_KP
                    pb, pqc = pit // N_QC, pit % N_QC
                    pq0 = pb * S + pqc * NF
                    if pkp == 0:
                        oA_new = ps.tile([HD + 1, NF], F32, tag="o", bufs=2)
                        oB_new = ps.tile([HD + 1, NF], F32, tag="o", bufs=2)
                        o_tiles[pit] = (oA_new, oB_new)
                    oA_ps, oB_ps = o_tiles[pit]
                    pt2A, pt2B = ptq.pop(pg)
                    for half in range(2):
                        kb = pkp * 2 + half
                        gkb = pb * N_KB + kb
                        nc.tensor.matmul(oA_ps[:], vA_sb[:, gkb], pt2A[:, half],
                                         start=(kb == 0), stop=(kb == N_KB - 1))
                        nc.tensor.matmul(oB_ps[:], vB_sb[:, gkb], pt2B[:, half],
                                         start=(kb == 0), stop=(kb == N_KB - 1))
                    if pkp == N_KP - 1:
                        # iteration finished: one PSUM read per head (with
                        # the rowsum row), then pack the rowsums from SBUF.
                        oA_raw = pt_pool.tile([HD + 1, NF], F32, tag="oraw", bufs=6)
                        oB_raw = pt_pool.tile([HD + 1, NF], F32, tag="oraw", bufs=6)
                        nc.vector.tensor_copy(oA_raw[:], oA_ps[:])
                        nc.vector.tensor_copy(oB_raw[:], oB_ps[:])
                        del o_tiles[pit]
                        zp = pt_pool.tile([HD + 1, NF], F32, tag="zp", bufs=2)
                        nc.vector.tensor_copy(zp[0:1, :], oA_raw[HD : HD + 1, :])
                        nc.vector.tensor_copy(zp[HD : HD + 1, :], oB_raw[HD : HD + 1, :])
                        pending = (oA_raw, oB_raw, zp, pq0, pit)

                def sec_filler():
                    nonlocal y_done
                    if g >= total_pairs:
                        return
                    spent = 0
                    while filler and filler[0][0] <= g + 2:
                        ent = filler.pop(0)
                        emit_filler(ent)
                        spent += ent[2]
                    target = 1500 if filler else 1100
                    # the kp==1 slot must bridge the previous iteration's
                    # PSUM evictions (the o-ring reuse), so it may draw the
                    # y reserve down further
                    floor = 2 if kp == 1 else 4
                    ny = 0
                    while spent < target:
                        if filler:
                            ent = filler.pop(0)
                            emit_filler(ent)
                            spent += ent[2]
                        elif y_avail and ny < 2 and len(y_avail) > floor:
                            emit_ypair(y_avail.pop(0))
                            y_done += 1
                            ny += 1
                            spent += 1024
                        else:
                            break

                # At kp==1 the previous slot's PV closed an iteration and its
                # PSUM evictions are still draining on DVE; the o-ring reuse
                # in this slot's PV would stall the in-order PE, so let the
                # filler run between ST and PV there. Everywhere else the PV
                # goes first so boundary evictions beat filler epilogues into
                # the DVE queue.
                sec_st()
                if kp == 1:
                    sec_filler()
                    sec_pv()
                else:
                    sec_pv()
                    sec_filler()

            # ---- drain: last iteration's normalization + remaining y.
            # The reserved y tiles are emitted BEFORE the apply (matmul
            # stationary reads of oT_sb are dependency-tracked coarsely, so
            # anything after the apply waits for it) and evict via ACT,
            # which is idle after the recip — the DVE queue stays clear for
            # the apply itself.
            norm_state = emit_recip(*pending)
            bc_state = emit_bcast(*norm_state)
            rest = y_avail[:]
            y_avail.clear()
            for s0 in rest:
                emit_ypair(s0, drain=True)
            emit_apply(*bc_state)        # appends the last iteration's tiles
            for s0 in y_avail:
                emit_ypair(s0, drain=True)

    nc.compile()
    return nc


def kernel(x, Wq, bq, Wk, bk, Wv, bv, Wo, bo, _trace=False):
    global last_results
    x = np.asarray(x, dtype=np.float32)
    Wq, bq = np.asarray(Wq, np.float32), np.asarray(bq, np.float32)
    Wk, bk = np.asarray(Wk, np.float32), np.asarray(bk, np.float32)
    Wv, bv = np.asarray(Wv, np.float32), np.asarray(bv, np.float32)
    Wo, bo = np.asarray(Wo, np.float32), np.asarray(bo, np.float32)

    if "nc" not in _cache:
        _cache["nc"] = _build()
    nc = _cache["nc"]

    dt_qk, dt_v, dt_out = _np_dt(DT_QK), _np_dt(DT_V), _np_dt(DT_OUT)
    # xs[p, sb, a, s'] = x[sb*NF+s', a*P+p] — per-partition contiguous strips
    x2 = x.reshape(BS, D)
    xs = np.ascontiguousarray(
        x2.reshape(N_SB, NF, N_DC, P).transpose(3, 0, 2, 1).reshape(P, -1)
    ).astype(dt_qk, copy=False)

    def chunk_w(W, sl, dt):
        # w[p, a, e] = W[sl][e, a*P+p]
        u = np.ascontiguousarray(W[sl].T).reshape(N_DC, P, ES)
        return np.ascontiguousarray(u.transpose(1, 0, 2).reshape(P, -1)).astype(dt, copy=False)

    in_maps = []
    for c in range(NCORES):
        sl = slice(c * ES, (c + 1) * ES)
        in_maps.append({
            "xs": xs,
            "wqc": chunk_w(Wq, sl, dt_qk),
            "wkc": chunk_w(Wk, sl, dt_qk),
            "wvc": chunk_w(Wv, sl, dt_v),
            "bq": np.ascontiguousarray(bq[sl, None]),
            "bk": np.ascontiguousarray(bk[sl, None]),
            "bv": np.ascontiguousarray(bv[None, sl]),
            "woT": np.ascontiguousarray(Wo[:, sl].T).astype(dt_out, copy=False),
        })

    res = bass_utils.run_bass_kernel_spmd(
        nc, in_maps, core_ids=list(range(NCORES)), trace=_trace)
    last_results = res

    y = res.results[0]["y"].astype(np.float64)
    for c in range(1, NCORES):
        y += res.results[c]["y"]
    y = (y + bo).astype(np.float32)
    return y.reshape(B, S, D)


# revision 37
# speedup vs baseline: 1.0369x; 1.0369x over previous
"""Chunked (= full, non-causal) multi-head self-attention on 8 TRN2 NeuronCores.

Problem: B=2, S=2048, D=1024, H=16 heads (head_dim 64), torch-Linear-style
projections (y = x @ W.T + b), softmax attention, output projection.

Sharding: head-parallel. Core c owns heads {2c, 2c+1} = feature slice
[128c, 128c+128). Each core computes q/k/v for its slice from the full x
(replicated), runs attention for its 4 (batch, head) pairs, and produces a
partial output projection with its 128-row slice of Wo. Host sums the 8
partials and adds bo.

Layout: scores are computed transposed, ST[k, q] (keys on partitions), so the
softmax exp output PT feeds the P@V matmul directly (contraction over k on
partitions) with no on-chip transposes anywhere — x and the weights are
pre-chunked on the host so every DMA moves fat contiguous per-partition
blocks. The softmax denominator rides as row 64 of the PV output via a
ones-column appended to V (M=65); normalization packs both heads' rowsums
into a [2, NF] tile so the reciprocal is ONE Ln + ONE Exp on ACT, emitted
into ACT's natural wait-for-scores bubble at the iteration boundary.

Scheduling: a single software-pipelined stream over all 64 (batch, qc, kp)
score/PV pair-slots. Attention starts as soon as sblock 0's q/k/v exist
(~5us); every remaining projection piece and output-projection tile is
metered into the stream as PE filler with per-piece deadlines, so the PE
never idles (TRN2's HAM clock-gate halves the PE clock when it does) and the
ACT exp stream never outruns it. A few y-tiles are held back to cover the
final normalization chain's latency at the drain.

Precision: everything bf16 in, fp32 accumulate/out (~5e-3 rel err).
"""

import sys

if "/opt/trn_rl_repo" not in sys.path:
    sys.path.insert(0, "/opt/trn_rl_repo")

import numpy as np

import concourse.bacc as bacc
import concourse.mybir as mybir
import concourse.tile as tile
from concourse import bass_utils

# Route Exp to the activation-table set that also holds Ln, so the softmax
# exps and the reciprocal-via-exp(-ln(x)) trick share one table (the default
# per-function choice would ping-pong table loads at ~2.7us each).
_orig_get_activation_tables = bacc.get_activation_tables


def _patched_get_activation_tables(arch):
    out = {}
    for name, funcs in dict(_orig_get_activation_tables(arch)).items():
        if name != "natural_log_exp_and_others":
            funcs = {f for f in funcs if f != mybir.ActivationFunctionType.Exp}
        out[name] = funcs
    return out


bacc.get_activation_tables = _patched_get_activation_tables

B, S, D, H = 2, 2048, 1024, 16
HD = D // H          # 64
NCORES = 8
ES = D // NCORES     # 128 features (= 2 heads) per core
BS = B * S           # 4096 rows total

P = 128              # partitions
NF = 512             # matmul free-dim tile
N_SB = BS // NF      # 8 s-blocks of 512
N_DC = D // P        # 8 contraction chunks of 128
N_KB = S // P        # 16 key blocks of 128 per batch
N_KP = N_KB // 2     # 8 key-block PAIRS per batch
N_QC = S // NF       # 4 query chunks of 512 per batch
N_CH = BS // P       # 32 global 128-row chunks

F32 = mybir.dt.float32
BF16 = mybir.dt.bfloat16

DT_QK = BF16         # x/Wq/Wk inputs for q,k projections + score matmuls
DT_V = BF16          # x/Wv inputs for v projection
DT_ATT = BF16        # attention weights (exp output) and V in the P@V matmul
DT_OUT = BF16        # output projection inputs (OT, Wo)

_cache = {}
last_results = None          # test.py reads exec_time_ns off this


def _np_dt(dt):
    import ml_dtypes

    return np.dtype(ml_dtypes.bfloat16) if dt == mybir.dt.bfloat16 else np.dtype(np.float32)


def _build():
    nc = bacc.Bacc("TRN2", target_bir_lowering=False, debug=False)

    # Host-pre-chunked layouts: per-partition-contiguous so every DMA uses
    # 128 fat descriptors instead of 1024 thin ones.
    xs_d = nc.dram_tensor("xs", [P, N_SB * N_DC * NF], DT_QK, kind="ExternalInput")
    wqc_d = nc.dram_tensor("wqc", [P, N_DC * ES], DT_QK, kind="ExternalInput")
    wkc_d = nc.dram_tensor("wkc", [P, N_DC * ES], DT_QK, kind="ExternalInput")
    wvc_d = nc.dram_tensor("wvc", [P, N_DC * ES], DT_V, kind="ExternalInput")
    bq_d = nc.dram_tensor("bq", [ES, 1], F32, kind="ExternalInput")
    bk_d = nc.dram_tensor("bk", [ES, 1], F32, kind="ExternalInput")
    bv_d = nc.dram_tensor("bv", [1, ES], F32, kind="ExternalInput")
    woT_d = nc.dram_tensor("woT", [ES, D], DT_OUT, kind="ExternalInput")
    y_d = nc.dram_tensor("y", [BS, D], F32, kind="ExternalOutput")

    xs_r = xs_d.ap().rearrange("p (b a s) -> p b a s", b=N_SB, a=N_DC)
    wq_r = wqc_d.ap().rearrange("p (a e) -> p a e", a=N_DC)
    wk_r = wkc_d.ap().rearrange("p (a e) -> p a e", a=N_DC)
    wv_r = wvc_d.ap().rearrange("p (a e) -> p a e", a=N_DC)

    with tile.TileContext(nc) as tc:
        with tc.tile_pool(name="const", bufs=1) as cpool, \
             tc.tile_pool(name="xt", bufs=4) as xt_pool, \
             tc.tile_pool(name="qkv", bufs=1) as qkv_pool, \
             tc.tile_pool(name="pt", bufs=14) as pt_pool, \
             tc.tile_pool(name="ysb", bufs=6) as y_pool, \
             tc.tile_pool(name="ps", bufs=1, space="PSUM") as ps:

            # ---- weights / constants ------------------------------------
            # The critical-path DMAs (strip0 for k0's moving data, then
            # wk/wq) go FIRST on the SP queue in need-order; everything else
            # rides the ACT queue so its DGE setup overlaps SP's.
            strips = {}

            def emit_strip_dma(sb):
                strip = xt_pool.tile([P, N_DC, NF], DT_QK, tag="strip", name=f"strip{sb}")
                nc.sync.dma_start(strip[:], xs_r[:, sb])
                strips[sb] = strip

            wk_sb = cpool.tile([P, N_DC, ES], DT_QK)
            wq_sb = cpool.tile([P, N_DC, ES], DT_QK)
            nc.sync.dma_start(wk_sb[:], wk_r[:, :, :])
            nc.sync.dma_start(wq_sb[:], wq_r[:, :, :])
            emit_strip_dma(0)
            emit_strip_dma(1)

            bq_sb = cpool.tile([ES, 1], F32)
            bk_sb = cpool.tile([ES, 1], F32)
            bv_row = cpool.tile([1, ES], F32)
            nc.scalar.dma_start(bq_sb[:], bq_d[:])
            nc.scalar.dma_start(bk_sb[:], bk_d[:])
            nc.scalar.dma_start(bv_row[:], bv_d[:])
            wv_sb = cpool.tile([P, N_DC, ES], DT_V)
            nc.scalar.dma_start(wv_sb[:], wv_r[:, :, :])
            wo_sb = cpool.tile([ES, D], DT_OUT)
            nc.scalar.dma_start(wo_sb[:], woT_d[:])

            ones_row = cpool.tile([1, ES], F32)
            nc.vector.memset(ones_row[:], 1.0)

            # ---- persistent activations ---------------------------------
            qT_sb = qkv_pool.tile([P, BS], DT_QK)     # [feat 128, s 4096]
            kT_sb = qkv_pool.tile([P, BS], DT_QK)
            vA_sb = qkv_pool.tile([P, N_CH, HD + 1], DT_ATT)  # head A V + ones col
            vB_sb = qkv_pool.tile([P, N_CH, HD + 1], DT_ATT)
            oT_sb = qkv_pool.tile([P, BS], DT_OUT)    # normalized attn out, [feat, s]
            nc.vector.memset(vA_sb[:, :, HD : HD + 1], 1.0)
            nc.vector.memset(vB_sb[:, :, HD : HD + 1], 1.0)
            # pre-touch the packed-rowsum ring (heads live at partitions 0 and
            # 64; the ACT Ln reads the whole [65, NF] block, so the unused
            # partitions must have been written once)
            for _ in range(2):
                zp_init = pt_pool.tile([HD + 1, NF], F32, tag="zp", bufs=2)
                nc.vector.memset(zp_init[:], 1.0)

            # ---- emission helpers ---------------------------------------
            def emit_qk_piece(sb, which):
                s0 = sb * NF
                strip = strips[sb]
                w_sb, bias, dst = ((wq_sb, bq_sb, qT_sb) if which == "q"
                                   else (wk_sb, bk_sb, kT_sb))
                p_ps = ps.tile([P, NF], F32, tag="misc", bufs=2, name=f"{which}{sb}_ps")
                for j in range(N_DC):
                    nc.tensor.matmul(p_ps[:], w_sb[:, j], strip[:, j],
                                     start=(j == 0), stop=(j == N_DC - 1))
                nc.vector.tensor_scalar_add(dst[:, s0 : s0 + NF], p_ps[:], bias[:])

            def emit_v_piece(sb, ss):
                strip = strips[sb]
                ch = sb * (NF // P) + ss
                v_ps = ps.tile([P, ES], F32, tag="misc", bufs=2, name=f"v{ch}_ps")
                for j in range(N_DC):
                    nc.tensor.matmul(v_ps[:], strip[:, j, ss * P : (ss + 1) * P],
                                     wv_sb[:, j],
                                     start=(j == 0), stop=(j == N_DC - 1))
                nc.vector.tensor_add(vA_sb[:, ch, 0:HD], v_ps[:, 0:HD], bv_bc[:, 0:HD])
                nc.vector.tensor_add(vB_sb[:, ch, 0:HD], v_ps[:, HD:ES], bv_bc[:, HD:ES])

            # y output: both 512-wide halves of one 128-row block in one go —
            # one SBUF tile, ONE dma (halves the SP queue's per-dma cost).
            # In-loop the two matmuls ride the misc PSUM ring with DVE
            # eviction; at the drain they use the freed st2 ring with the
            # evictions split across DVE and ACT so the PE never waits.
            drain_dma = [0]

            def emit_ypair(s0, drain=False):
                y2_sb = y_pool.tile([P, 2, NF], F32, tag="y")
                if drain:
                    y_ps = ps.tile([P, 2, NF], F32, tag="st2", bufs=2)
                    for ec in range(2):
                        nc.tensor.matmul(y_ps[:, ec], oT_sb[:, s0 : s0 + P],
                                         wo_sb[:, ec * NF : (ec + 1) * NF],
                                         start=True, stop=True)
                    nc.vector.tensor_copy(y2_sb[:, 0], y_ps[:, 0])
                    nc.scalar.activation(y2_sb[:, 1], y_ps[:, 1],
                                         mybir.ActivationFunctionType.Copy)
                    eng = nc.sync if drain_dma[0] % 2 == 0 else nc.scalar
                    drain_dma[0] += 1
                    eng.dma_start(y_d[s0 : s0 + P, :], y2_sb.rearrange("p a b -> p (a b)"))
                else:
                    for ec in range(2):
                        y_ps = ps.tile([P, NF], F32, tag="misc", bufs=2)
                        nc.tensor.matmul(y_ps[:], oT_sb[:, s0 : s0 + P],
                                         wo_sb[:, ec * NF : (ec + 1) * NF],
                                         start=True, stop=True)
                        nc.vector.tensor_copy(y2_sb[:, ec], y_ps[:])
                    nc.sync.dma_start(y_d[s0 : s0 + P, :], y2_sb.rearrange("p a b -> p (a b)"))

            # ---- filler plan (deadline = pair index it must be done by) --
            # pair g = b*32 + qc*8 + kp. k(sb)/v(sb,ss) feed the kp sweep of
            # EVERY qc iteration of batch sb//4, so they're due at qc=0's
            # consuming pair; q(sb) is due when iteration (b, qc=sb%4) starts.
            filler = []            # (deadline, kind, cost_cycles, args)

            def dl_k(sb):
                return (sb // 4) * 32 + (sb % 4) * 2

            def dl_v(sb, ss):
                return (sb // 4) * 32 + ((sb % 4) * 4 + ss) // 2

            def dl_q(sb):
                return (sb // 4) * 32 + (sb % 4) * 8

            for sb in (1, 2, 3):
                filler.append((max(0, dl_k(sb) - 1), "dma", 0, (sb + 1,)))
                filler.append((dl_k(sb), "k", 4096, (sb,)))
                for ss in range(4):
                    filler.append((dl_v(sb, ss), "v", 1024, (sb, ss)))
            filler.append((8, "q", 4096, (1,)))
            filler.append((16, "q", 4096, (2,)))
            filler.append((24, "q", 4096, (3,)))
            for sb in (4, 5, 6, 7):
                filler.append((dl_k(sb) - 4, "dma", 0, (sb,)))
                filler.append((dl_k(sb), "k", 4096, (sb,)))
                if sb == 4:
                    filler.append((dl_q(sb), "q", 4096, (sb,)))
                for ss in range(4):
                    filler.append((dl_v(sb, ss), "v", 1024, (sb, ss)))
            for sb in (5, 6, 7):
                filler.append((dl_q(sb), "q", 4096, (sb,)))
            filler.sort(key=lambda e: e[0])

            bv_bc_done = [False]

            def emit_filler(ent):
                _, kind, _, args = ent
                if kind == "dma":
                    emit_strip_dma(*args)
                elif kind == "k":
                    emit_qk_piece(args[0], "k")
                elif kind == "q":
                    emit_qk_piece(args[0], "q")
                else:
                    if not bv_bc_done[0]:
                        emit_bv_bc()
                        bv_bc_done[0] = True
                    emit_v_piece(*args)

            # ---- startup: k0/q0 first so the first scores (and thus ACT's
            # exp stream) start as early as possible; bv_bc and sblock 0's v
            # pieces follow as pair-0/1 filler.
            emit_qk_piece(0, "k")
            emit_qk_piece(0, "q")

            bv_bc = cpool.tile([P, ES], F32)

            def emit_bv_bc():
                # bv broadcast to all 128 partitions via rank-1 matmul
                bv_bc_ps = ps.tile([P, ES], F32, tag="misc", bufs=2)
                nc.tensor.matmul(bv_bc_ps[:], ones_row[:], bv_row[:],
                                 start=True, stop=True)
                nc.vector.tensor_copy(bv_bc[:], bv_bc_ps[:])

            for ss in range(4):
                filler.insert(ss, (0, "v", 1024, (0, ss)))

            inv_sqrt_hd = 1.0 / float(np.sqrt(HD))

            # ---- attention: one continuous software-pipelined stream -----
            n_iters = B * N_QC
            total_pairs = n_iters * N_KP
            o_tiles = {}
            ptq = {}
            pending = None           # iter finished, awaiting recip chain
            norm_state = None        # recip done, awaiting broadcast+apply
            bc_state = None
            y_avail = []             # y tiles whose apply has been emitted
            y_done = 0

            def emit_recip(oA_raw, oB_raw, zp, q0, it):
                # 1/rowsum via the single-op DVE Newton-Raphson reciprocal
                # (~18 bits — ample for softmax normalization; the rowsums
                # are benign positive values). Keeping the whole chain on
                # DVE->Pool->DVE leaves ACT's exp stream uninterrupted.
                zr = pt_pool.tile([HD + 1, NF], F32, tag="zr", bufs=2)
                nc.vector.reciprocal_approx_fast(zr[:], zp[:])
                return (oA_raw, oB_raw, zr, q0, it)

            def emit_bcast(oA_raw, oB_raw, zr, q0, it):
                # partition_broadcast always sources the tile's partition 0,
                # so head B's recip (at partition 64) is de-packed first. The
                # de-pack runs on DVE: the Pool engine must execute ONLY
                # partition_broadcast — any other op type forces a ~7us
                # GPSIMD ucode library swap (MODIFY_POOL_CONFIG).
                zrB = pt_pool.tile([1, NF], F32, tag="zrB", bufs=2)
                nc.vector.tensor_copy(zrB[:], zr[HD : HD + 1, :])
                bc2 = pt_pool.tile([HD, 2, NF], F32, tag="bc", bufs=3)
                nc.gpsimd.partition_broadcast(bc2[:, 0], zr[0:1, :])
                nc.gpsimd.partition_broadcast(bc2[:, 1], zrB[:])
                return (oA_raw, oB_raw, bc2, q0, it)

            def emit_apply(oA_raw, oB_raw, bc2, q0, it):
                for hidx, (o_raw, part) in enumerate(((oA_raw, 0), (oB_raw, HD))):
                    nc.vector.tensor_mul(
                        oT_sb[part : part + HD, q0 : q0 + NF],
                        o_raw[0:HD, :], bc2[:, hidx])
                for ss in range(NF // P):
                    y_avail.append(q0 + ss * P)

            for g in range(total_pairs + 1):
                kp = g % N_KP
                # iteration-boundary chain (pending is set at the kp==0 slot,
                # when the previous iteration's last PV lands), emitted at
                # slot start so the ACT ops land before this slot's exps and
                # the Pool/DVE ops run under the PE's score matmuls.
                if kp == 1 and pending is not None:
                    norm_state = emit_recip(*pending)
                    pending = None
                if kp == 2 and norm_state is not None:
                    bc_state = emit_bcast(*norm_state)
                    norm_state = None
                if kp == 3 and bc_state is not None:
                    emit_apply(*bc_state)
                    bc_state = None

                def sec_st():
                    if g >= total_pairs:
                        return
                    it = g // N_KP
                    b, qc = it // N_QC, it % N_QC
                    q0 = b * S + qc * NF
                    st2A = ps.tile([P, 2, NF], F32, tag="st2", bufs=2)
                    st2B = ps.tile([P, 2, NF], F32, tag="st2", bufs=2)
                    for half in range(2):
                        k0 = b * S + (kp * 2 + half) * P
                        nc.tensor.matmul(st2A[:, half], kT_sb[0:HD, k0 : k0 + P],
                                         qT_sb[0:HD, q0 : q0 + NF],
                                         start=True, stop=True)
                        nc.tensor.matmul(st2B[:, half], kT_sb[HD:P, k0 : k0 + P],
                                         qT_sb[HD:P, q0 : q0 + NF],
                                         start=True, stop=True)
                    pt2A = pt_pool.tile([P, 2, NF], DT_ATT, tag="pt", bufs=14)
                    pt2B = pt_pool.tile([P, 2, NF], DT_ATT, tag="pt", bufs=14)
                    nc.scalar.activation(pt2A[:], st2A[:],
                                         mybir.ActivationFunctionType.Exp,
                                         scale=inv_sqrt_hd)
                    nc.scalar.activation(pt2B[:], st2B[:],
                                         mybir.ActivationFunctionType.Exp,
                                         scale=inv_sqrt_hd)
                    ptq[g] = (pt2A, pt2B)

                def sec_pv():
                    nonlocal pending
                    if g < 1:
                        return
                    pg = g - 1
                    pit = pg // N_KP
                    pkp = pg % N_KP
                    pb, pqc = pit // N_QC, pit % N_QC
                    pq0 = pb * S + pqc * NF
                    if pkp == 0:
                        oA_new = ps.tile([HD + 1, NF], F32, tag="o", bufs=2)
                        oB_new = ps.tile([HD + 1, NF], F32, tag="o", bufs=2)
                        o_tiles[pit] = (oA_new, oB_new)
                    oA_ps, oB_ps = o_tiles[pit]
                    pt2A, pt2B = ptq.pop(pg)
                    for half in range(2):
                        kb = pkp * 2 + half
                        gkb = pb * N_KB + kb
                        nc.tensor.matmul(oA_ps[:], vA_sb[:, gkb], pt2A[:, half],
                                         start=(kb == 0), stop=(kb == N_KB - 1))
                        nc.tensor.matmul(oB_ps[:], vB_sb[:, gkb], pt2B[:, half],
                                         start=(kb == 0), stop=(kb == N_KB - 1))
                    if pkp == N_KP - 1:
                        # iteration finished: one PSUM read per head (with
                        # the rowsum row), then pack the rowsums from SBUF.
                        oA_raw = pt_pool.tile([HD + 1, NF], F32, tag="oraw", bufs=6)
                        oB_raw = pt_pool.tile([HD + 1, NF], F32, tag="oraw", bufs=6)
                        nc.vector.tensor_copy(oA_raw[:], oA_ps[:])
                        nc.vector.tensor_copy(oB_raw[:], oB_ps[:])
                        del o_tiles[pit]
                        zp = pt_pool.tile([HD + 1, NF], F32, tag="zp", bufs=2)
                        nc.vector.tensor_copy(zp[0:1, :], oA_raw[HD : HD + 1, :])
                        nc.vector.tensor_copy(zp[HD : HD + 1, :], oB_raw[HD : HD + 1, :])
                        pending = (oA_raw, oB_raw, zp, pq0, pit)

                def sec_filler():
                    nonlocal y_done
                    if g >= total_pairs:
                        return
                    spent = 0
                    while filler and filler[0][0] <= g + 2:
                        ent = filler.pop(0)
                        emit_filler(ent)
                        spent += ent[2]
                    target = 1500 if filler else 1100
                    # the kp==1 slot must bridge the previous iteration's
                    # PSUM evictions (the o-ring reuse), so it may draw the
                    # y reserve down further
                    floor = 2 if kp == 1 else 4
                    ny = 0
                    while spent < target:
                        if filler:
                            ent = filler.pop(0)
                            emit_filler(ent)
                            spent += ent[2]
                        elif y_avail and ny < 2 and len(y_avail) > floor:
                            emit_ypair(y_avail.pop(0))
                            y_done += 1
                            ny += 1
                            spent += 1024
                        else:
                            break

                # At kp==1 the previous slot's PV closed an iteration and its
                # PSUM evictions are still draining on DVE; the o-ring reuse
                # in this slot's PV would stall the in-order PE, so let the
                # filler run between ST and PV there. Everywhere else the PV
                # goes first so boundary evictions beat filler epilogues into
                # the DVE queue.
                sec_st()
                if kp == 1:
                    sec_filler()
                    sec_pv()
                else:
                    sec_pv()
                    sec_filler()

            # ---- drain: last iteration's normalization + remaining y.
            # The reserved y tiles are emitted BEFORE the apply (matmul
            # stationary reads of oT_sb are dependency-tracked coarsely, so
            # anything after the apply waits for it) and evict via ACT,
            # which is idle after the recip — the DVE queue stays clear for
            # the apply itself.
            norm_state = emit_recip(*pending)
            bc_state = emit_bcast(*norm_state)
            rest = y_avail[:]
            y_avail.clear()
            for s0 in rest:
                emit_ypair(s0, drain=True)
            emit_apply(*bc_state)        # appends the last iteration's tiles
            for s0 in y_avail:
                emit_ypair(s0, drain=True)

    nc.compile()
    return nc


def kernel(x, Wq, bq, Wk, bk, Wv, bv, Wo, bo, _trace=False):
    global last_results
    x = np.asarray(x, dtype=np.float32)
    Wq, bq = np.asarray(Wq, np.float32), np.asarray(bq, np.float32)
    Wk, bk = np.asarray(Wk, np.float32), np.asarray(bk, np.float32)
    Wv, bv = np.asarray(Wv, np.float32), np.asarray(bv, np.float32)
    Wo, bo = np.asarray(Wo, np.float32), np.asarray(bo, np.float32)

    if "nc" not in _cache:
        _cache["nc"] = _build()
    nc = _cache["nc"]

    dt_qk, dt_v, dt_out = _np_dt(DT_QK), _np_dt(DT_V), _np_dt(DT_OUT)
    # xs[p, sb, a, s'] = x[sb*NF+s', a*P+p] — per-partition contiguous strips
    x2 = x.reshape(BS, D)
    xs = np.ascontiguousarray(
        x2.reshape(N_SB, NF, N_DC, P).transpose(3, 0, 2, 1).reshape(P, -1)
    ).astype(dt_qk, copy=False)

    def chunk_w(W, sl, dt):
        # w[p, a, e] = W[sl][e, a*P+p]
        u = np.ascontiguousarray(W[sl].T).reshape(N_DC, P, ES)
        return np.ascontiguousarray(u.transpose(1, 0, 2).reshape(P, -1)).astype(dt, copy=False)

    in_maps = []
    for c in range(NCORES):
        sl = slice(c * ES, (c + 1) * ES)
        in_maps.append({
            "xs": xs,
            "wqc": chunk_w(Wq, sl, dt_qk),
            "wkc": chunk_w(Wk, sl, dt_qk),
            "wvc": chunk_w(Wv, sl, dt_v),
            "bq": np.ascontiguousarray(bq[sl, None]),
            "bk": np.ascontiguousarray(bk[sl, None]),
            "bv": np.ascontiguousarray(bv[None, sl]),
            "woT": np.ascontiguousarray(Wo[:, sl].T).astype(dt_out, copy=False),
        })

    res = bass_utils.run_bass_kernel_spmd(
        nc, in_maps, core_ids=list(range(NCORES)), trace=_trace)
    last_results = res

    y = res.results[0]["y"].astype(np.float64)
    for c in range(1, NCORES):
        y += res.results[c]["y"]
    y = (y + bo).astype(np.float32)
    return y.reshape(B, S, D)


# revision 39
# speedup vs baseline: 1.0535x; 1.0160x over previous
"""Chunked (= full, non-causal) multi-head self-attention on 8 TRN2 NeuronCores.

Problem: B=2, S=2048, D=1024, H=16 heads (head_dim 64), torch-Linear-style
projections (y = x @ W.T + b), softmax attention, output projection.

Sharding: head-parallel. Core c owns heads {2c, 2c+1} = feature slice
[128c, 128c+128). Each core computes q/k/v for its slice from the full x
(replicated), runs attention for its 4 (batch, head) pairs, and produces a
partial output projection with its 128-row slice of Wo. Host sums the 8
partials and adds bo.

Layout: scores are computed transposed, ST[k, q] (keys on partitions), so the
softmax exp output PT feeds the P@V matmul directly (contraction over k on
partitions) with no on-chip transposes anywhere — x and the weights are
pre-chunked on the host so every DMA moves fat contiguous per-partition
blocks. The softmax denominator rides as row 64 of the PV output via a
ones-column appended to V (M=65); normalization packs both heads' rowsums
into a [2, NF] tile so the reciprocal is ONE Ln + ONE Exp on ACT, emitted
into ACT's natural wait-for-scores bubble at the iteration boundary.

Scheduling: a single software-pipelined stream over all 64 (batch, qc, kp)
score/PV pair-slots. Attention starts as soon as sblock 0's q/k/v exist
(~5us); every remaining projection piece and output-projection tile is
metered into the stream as PE filler with per-piece deadlines, so the PE
never idles (TRN2's HAM clock-gate halves the PE clock when it does) and the
ACT exp stream never outruns it. A few y-tiles are held back to cover the
final normalization chain's latency at the drain.

Precision: everything bf16 in, fp32 accumulate/out (~5e-3 rel err).
"""

import sys

if "/opt/trn_rl_repo" not in sys.path:
    sys.path.insert(0, "/opt/trn_rl_repo")

import numpy as np

import concourse.bacc as bacc
import concourse.mybir as mybir
import concourse.tile as tile
from concourse import bass_utils

# Route Exp to the activation-table set that also holds Ln, so the softmax
# exps and the reciprocal-via-exp(-ln(x)) trick share one table (the default
# per-function choice would ping-pong table loads at ~2.7us each).
_orig_get_activation_tables = bacc.get_activation_tables


def _patched_get_activation_tables(arch):
    out = {}
    for name, funcs in dict(_orig_get_activation_tables(arch)).items():
        if name != "natural_log_exp_and_others":
            funcs = {f for f in funcs if f != mybir.ActivationFunctionType.Exp}
        out[name] = funcs
    return out


bacc.get_activation_tables = _patched_get_activation_tables

B, S, D, H = 2, 2048, 1024, 16
HD = D // H          # 64
NCORES = 8
ES = D // NCORES     # 128 features (= 2 heads) per core
BS = B * S           # 4096 rows total

P = 128              # partitions
NF = 512             # matmul free-dim tile
N_SB = BS // NF      # 8 s-blocks of 512
N_DC = D // P        # 8 contraction chunks of 128
N_KB = S // P        # 16 key blocks of 128 per batch
N_KP = N_KB // 2     # 8 key-block PAIRS per batch
N_QC = S // NF       # 4 query chunks of 512 per batch
N_CH = BS // P       # 32 global 128-row chunks

F32 = mybir.dt.float32
BF16 = mybir.dt.bfloat16

DT_QK = BF16         # x/Wq/Wk inputs for q,k projections + score matmuls
DT_V = BF16          # x/Wv inputs for v projection
DT_ATT = BF16        # attention weights (exp output) and V in the P@V matmul
DT_OUT = BF16        # output projection inputs (OT, Wo)

_cache = {}
last_results = None          # test.py reads exec_time_ns off this


def _np_dt(dt):
    import ml_dtypes

    return np.dtype(ml_dtypes.bfloat16) if dt == mybir.dt.bfloat16 else np.dtype(np.float32)


def _build():
    nc = bacc.Bacc("TRN2", target_bir_lowering=False, debug=False)

    # Host-pre-chunked layouts: per-partition-contiguous so every DMA uses
    # 128 fat descriptors instead of 1024 thin ones.
    xs_d = nc.dram_tensor("xs", [P, N_SB * N_DC * NF], DT_QK, kind="ExternalInput")
    wqc_d = nc.dram_tensor("wqc", [P, N_DC * ES], DT_QK, kind="ExternalInput")
    wkc_d = nc.dram_tensor("wkc", [P, N_DC * ES], DT_QK, kind="ExternalInput")
    wvc_d = nc.dram_tensor("wvc", [P, N_DC * ES], DT_V, kind="ExternalInput")
    bq_d = nc.dram_tensor("bq", [ES, 1], F32, kind="ExternalInput")
    bk_d = nc.dram_tensor("bk", [ES, 1], F32, kind="ExternalInput")
    bv_d = nc.dram_tensor("bv", [1, ES], F32, kind="ExternalInput")
    woT_d = nc.dram_tensor("woT", [ES, D], DT_OUT, kind="ExternalInput")
    y_d = nc.dram_tensor("y", [BS, D], F32, kind="ExternalOutput")

    xs_r = xs_d.ap().rearrange("p (b a s) -> p b a s", b=N_SB, a=N_DC)
    wq_r = wqc_d.ap().rearrange("p (a e) -> p a e", a=N_DC)
    wk_r = wkc_d.ap().rearrange("p (a e) -> p a e", a=N_DC)
    wv_r = wvc_d.ap().rearrange("p (a e) -> p a e", a=N_DC)

    with tile.TileContext(nc) as tc:
        with tc.tile_pool(name="const", bufs=1) as cpool, \
             tc.tile_pool(name="xt", bufs=4) as xt_pool, \
             tc.tile_pool(name="qkv", bufs=1) as qkv_pool, \
             tc.tile_pool(name="pt", bufs=14) as pt_pool, \
             tc.tile_pool(name="ysb", bufs=6) as y_pool, \
             tc.tile_pool(name="ps", bufs=1, space="PSUM") as ps:

            # ---- weights / constants ------------------------------------
            # The critical-path DMAs (strip0 for k0's moving data, then
            # wk/wq) go FIRST on the SP queue in need-order; everything else
            # rides the ACT queue so its DGE setup overlaps SP's.
            strips = {}

            def emit_strip_dma(sb):
                strip = xt_pool.tile([P, N_DC, NF], DT_QK, tag="strip", name=f"strip{sb}")
                nc.sync.dma_start(strip[:], xs_r[:, sb])
                strips[sb] = strip

            wk_sb = cpool.tile([P, N_DC, ES], DT_QK)
            wq_sb = cpool.tile([P, N_DC, ES], DT_QK)
            nc.sync.dma_start(wk_sb[:], wk_r[:, :, :])
            nc.scalar.dma_start(wq_sb[:], wq_r[:, :, :])
            # strip0 is the critical-path transfer: split it across the SP
            # and ACT DMA queues so both halves fly in parallel.
            strip0 = xt_pool.tile([P, N_DC, NF], DT_QK, tag="strip", name="strip0")
            nc.sync.dma_start(strip0[:, 0 : N_DC // 2], xs_r[:, 0, 0 : N_DC // 2])
            nc.scalar.dma_start(strip0[:, N_DC // 2 : N_DC], xs_r[:, 0, N_DC // 2 : N_DC])
            strips[0] = strip0
            emit_strip_dma(1)

            bq_sb = cpool.tile([ES, 1], F32)
            bk_sb = cpool.tile([ES, 1], F32)
            bv_row = cpool.tile([1, ES], F32)
            nc.scalar.dma_start(bq_sb[:], bq_d[:])
            nc.scalar.dma_start(bk_sb[:], bk_d[:])
            nc.scalar.dma_start(bv_row[:], bv_d[:])
            wv_sb = cpool.tile([P, N_DC, ES], DT_V)
            nc.scalar.dma_start(wv_sb[:], wv_r[:, :, :])
            wo_sb = cpool.tile([ES, D], DT_OUT)
            nc.scalar.dma_start(wo_sb[:], woT_d[:])

            ones_row = cpool.tile([1, ES], F32)
            nc.vector.memset(ones_row[:], 1.0)

            # ---- persistent activations ---------------------------------
            qT_sb = qkv_pool.tile([P, BS], DT_QK)     # [feat 128, s 4096]
            kT_sb = qkv_pool.tile([P, BS], DT_QK)
            vA_sb = qkv_pool.tile([P, N_CH, HD + 1], DT_ATT)  # head A V + ones col
            vB_sb = qkv_pool.tile([P, N_CH, HD + 1], DT_ATT)
            oT_sb = qkv_pool.tile([P, BS], DT_OUT)    # normalized attn out, [feat, s]
            nc.vector.memset(vA_sb[:, :, HD : HD + 1], 1.0)
            nc.vector.memset(vB_sb[:, :, HD : HD + 1], 1.0)
            # pre-touch the packed-rowsum ring (heads live at partitions 0 and
            # 64; the ACT Ln reads the whole [65, NF] block, so the unused
            # partitions must have been written once)
            for _ in range(2):
                zp_init = pt_pool.tile([HD + 1, NF], F32, tag="zp", bufs=2)
                nc.vector.memset(zp_init[:], 1.0)

            # ---- emission helpers ---------------------------------------
            def emit_qk_piece(sb, which):
                s0 = sb * NF
                strip = strips[sb]
                w_sb, bias, dst = ((wq_sb, bq_sb, qT_sb) if which == "q"
                                   else (wk_sb, bk_sb, kT_sb))
                p_ps = ps.tile([P, NF], F32, tag="misc", bufs=2, name=f"{which}{sb}_ps")
                for j in range(N_DC):
                    nc.tensor.matmul(p_ps[:], w_sb[:, j], strip[:, j],
                                     start=(j == 0), stop=(j == N_DC - 1))
                nc.vector.tensor_scalar_add(dst[:, s0 : s0 + NF], p_ps[:], bias[:])

            def emit_v_piece(sb, ss):
                strip = strips[sb]
                ch = sb * (NF // P) + ss
                v_ps = ps.tile([P, ES], F32, tag="misc", bufs=2, name=f"v{ch}_ps")
                for j in range(N_DC):
                    nc.tensor.matmul(v_ps[:], strip[:, j, ss * P : (ss + 1) * P],
                                     wv_sb[:, j],
                                     start=(j == 0), stop=(j == N_DC - 1))
                nc.vector.tensor_add(vA_sb[:, ch, 0:HD], v_ps[:, 0:HD], bv_bc[:, 0:HD])
                nc.vector.tensor_add(vB_sb[:, ch, 0:HD], v_ps[:, HD:ES], bv_bc[:, HD:ES])

            # y output: both 512-wide halves of one 128-row block in one go —
            # one SBUF tile, ONE dma (halves the SP queue's per-dma cost).
            # In-loop the two matmuls ride the misc PSUM ring with DVE
            # eviction; at the drain they use the freed st2 ring with the
            # evictions split across DVE and ACT so the PE never waits.
            drain_dma = [0]

            def emit_ypair(s0, drain=False):
                y2_sb = y_pool.tile([P, 2, NF], F32, tag="y")
                if drain:
                    y_ps = ps.tile([P, 2, NF], F32, tag="st2", bufs=2)
                    for ec in range(2):
                        nc.tensor.matmul(y_ps[:, ec], oT_sb[:, s0 : s0 + P],
                                         wo_sb[:, ec * NF : (ec + 1) * NF],
                                         start=True, stop=True)
                    nc.vector.tensor_copy(y2_sb[:, 0], y_ps[:, 0])
                    nc.scalar.activation(y2_sb[:, 1], y_ps[:, 1],
                                         mybir.ActivationFunctionType.Copy)
                    eng = nc.sync if drain_dma[0] % 2 == 0 else nc.scalar
                    drain_dma[0] += 1
                    eng.dma_start(y_d[s0 : s0 + P, :], y2_sb.rearrange("p a b -> p (a b)"))
                else:
                    for ec in range(2):
                        y_ps = ps.tile([P, NF], F32, tag="misc", bufs=2)
                        nc.tensor.matmul(y_ps[:], oT_sb[:, s0 : s0 + P],
                                         wo_sb[:, ec * NF : (ec + 1) * NF],
                                         start=True, stop=True)
                        nc.vector.tensor_copy(y2_sb[:, ec], y_ps[:])
                    nc.sync.dma_start(y_d[s0 : s0 + P, :], y2_sb.rearrange("p a b -> p (a b)"))

            # ---- filler plan (deadline = pair index it must be done by) --
            # pair g = b*32 + qc*8 + kp. k(sb)/v(sb,ss) feed the kp sweep of
            # EVERY qc iteration of batch sb//4, so they're due at qc=0's
            # consuming pair; q(sb) is due when iteration (b, qc=sb%4) starts.
            filler = []            # (deadline, kind, cost_cycles, args)

            def dl_k(sb):
                return (sb // 4) * 32 + (sb % 4) * 2

            def dl_v(sb, ss):
                return (sb // 4) * 32 + ((sb % 4) * 4 + ss) // 2

            def dl_q(sb):
                return (sb // 4) * 32 + (sb % 4) * 8

            for sb in (1, 2, 3):
                filler.append((max(0, dl_k(sb) - 1), "dma", 0, (sb + 1,)))
                filler.append((dl_k(sb), "k", 4096, (sb,)))
                for ss in range(4):
                    filler.append((dl_v(sb, ss), "v", 1024, (sb, ss)))
            filler.append((8, "q", 4096, (1,)))
            filler.append((16, "q", 4096, (2,)))
            filler.append((24, "q", 4096, (3,)))
            for sb in (4, 5, 6, 7):
                filler.append((dl_k(sb) - 4, "dma", 0, (sb,)))
                filler.append((dl_k(sb), "k", 4096, (sb,)))
                if sb == 4:
                    filler.append((dl_q(sb), "q", 4096, (sb,)))
                for ss in range(4):
                    filler.append((dl_v(sb, ss), "v", 1024, (sb, ss)))
            for sb in (5, 6, 7):
                filler.append((dl_q(sb), "q", 4096, (sb,)))
            filler.sort(key=lambda e: e[0])

            bv_bc_done = [False]

            def emit_filler(ent):
                _, kind, _, args = ent
                if kind == "dma":
                    emit_strip_dma(*args)
                elif kind == "k":
                    emit_qk_piece(args[0], "k")
                elif kind == "q":
                    emit_qk_piece(args[0], "q")
                else:
                    if not bv_bc_done[0]:
                        emit_bv_bc()
                        bv_bc_done[0] = True
                    emit_v_piece(*args)

            # ---- startup: k0/q0 first so the first scores (and thus ACT's
            # exp stream) start as early as possible; bv_bc and sblock 0's v
            # pieces follow as pair-0/1 filler.
            emit_qk_piece(0, "k")
            emit_qk_piece(0, "q")

            bv_bc = cpool.tile([P, ES], F32)

            def emit_bv_bc():
                # bv broadcast to all 128 partitions via rank-1 matmul
                bv_bc_ps = ps.tile([P, ES], F32, tag="misc", bufs=2)
                nc.tensor.matmul(bv_bc_ps[:], ones_row[:], bv_row[:],
                                 start=True, stop=True)
                nc.vector.tensor_copy(bv_bc[:], bv_bc_ps[:])

            for ss in range(4):
                filler.insert(ss, (0, "v", 1024, (0, ss)))

            inv_sqrt_hd = 1.0 / float(np.sqrt(HD))

            # ---- attention: one continuous software-pipelined stream -----
            n_iters = B * N_QC
            total_pairs = n_iters * N_KP
            o_tiles = {}
            ptq = {}
            pending = None           # iter finished, awaiting recip chain
            norm_state = None        # recip done, awaiting broadcast+apply
            bc_state = None
            y_avail = []             # y tiles whose apply has been emitted
            y_done = 0

            def emit_recip(oA_raw, oB_raw, zp, q0, it):
                # 1/rowsum via the single-op DVE Newton-Raphson reciprocal
                # (~18 bits — ample for softmax normalization; the rowsums
                # are benign positive values). Keeping the whole chain on
                # DVE->Pool->DVE leaves ACT's exp stream uninterrupted.
                zr = pt_pool.tile([HD + 1, NF], F32, tag="zr", bufs=2)
                nc.vector.reciprocal_approx_fast(zr[:], zp[:])
                return (oA_raw, oB_raw, zr, q0, it)

            def emit_bcast(oA_raw, oB_raw, zr, q0, it):
                # partition_broadcast always sources the tile's partition 0,
                # so head B's recip (at partition 64) is de-packed first. The
                # de-pack runs on DVE: the Pool engine must execute ONLY
                # partition_broadcast — any other op type forces a ~7us
                # GPSIMD ucode library swap (MODIFY_POOL_CONFIG).
                zrB = pt_pool.tile([1, NF], F32, tag="zrB", bufs=2)
                nc.vector.tensor_copy(zrB[:], zr[HD : HD + 1, :])
                bc2 = pt_pool.tile([HD, 2, NF], F32, tag="bc", bufs=3)
                nc.gpsimd.partition_broadcast(bc2[:, 0], zr[0:1, :])
                nc.gpsimd.partition_broadcast(bc2[:, 1], zrB[:])
                return (oA_raw, oB_raw, bc2, q0, it)

            def emit_apply(oA_raw, oB_raw, bc2, q0, it):
                for hidx, (o_raw, part) in enumerate(((oA_raw, 0), (oB_raw, HD))):
                    nc.vector.tensor_mul(
                        oT_sb[part : part + HD, q0 : q0 + NF],
                        o_raw[0:HD, :], bc2[:, hidx])
                for ss in range(NF // P):
                    y_avail.append(q0 + ss * P)

            for g in range(total_pairs + 1):
                kp = g % N_KP
                # iteration-boundary chain (pending is set at the kp==0 slot,
                # when the previous iteration's last PV lands), emitted at
                # slot start so the ACT ops land before this slot's exps and
                # the Pool/DVE ops run under the PE's score matmuls.
                if kp == 1 and pending is not None:
                    norm_state = emit_recip(*pending)
                    pending = None
                if kp == 2 and norm_state is not None:
                    bc_state = emit_bcast(*norm_state)
                    norm_state = None
                if kp == 3 and bc_state is not None:
                    emit_apply(*bc_state)
                    bc_state = None

                def sec_st():
                    if g >= total_pairs:
                        return
                    it = g // N_KP
                    b, qc = it // N_QC, it % N_QC
                    q0 = b * S + qc * NF
                    st2A = ps.tile([P, 2, NF], F32, tag="st2", bufs=2)
                    st2B = ps.tile([P, 2, NF], F32, tag="st2", bufs=2)
                    for half in range(2):
                        k0 = b * S + (kp * 2 + half) * P
                        nc.tensor.matmul(st2A[:, half], kT_sb[0:HD, k0 : k0 + P],
                                         qT_sb[0:HD, q0 : q0 + NF],
                                         start=True, stop=True)
                        nc.tensor.matmul(st2B[:, half], kT_sb[HD:P, k0 : k0 + P],
                                         qT_sb[HD:P, q0 : q0 + NF],
                                         start=True, stop=True)
                    pt2A = pt_pool.tile([P, 2, NF], DT_ATT, tag="pt", bufs=14)
                    pt2B = pt_pool.tile([P, 2, NF], DT_ATT, tag="pt", bufs=14)
                    nc.scalar.activation(pt2A[:], st2A[:],
                                         mybir.ActivationFunctionType.Exp,
                                         scale=inv_sqrt_hd)
                    nc.scalar.activation(pt2B[:], st2B[:],
                                         mybir.ActivationFunctionType.Exp,
                                         scale=inv_sqrt_hd)
                    ptq[g] = (pt2A, pt2B)

                def sec_pv():
                    nonlocal pending
                    if g < 1:
                        return
                    pg = g - 1
                    pit = pg // N_KP
                    pkp = pg % N_KP
                    pb, pqc = pit // N_QC, pit % N_QC
                    pq0 = pb * S + pqc * NF
                    if pkp == 0:
                        oA_new = ps.tile([HD + 1, NF], F32, tag="o", bufs=2)
                        oB_new = ps.tile([HD + 1, NF], F32, tag="o", bufs=2)
                        o_tiles[pit] = (oA_new, oB_new)
                    oA_ps, oB_ps = o_tiles[pit]
                    pt2A, pt2B = ptq.pop(pg)
                    for half in range(2):
                        kb = pkp * 2 + half
                        gkb = pb * N_KB + kb
                        nc.tensor.matmul(oA_ps[:], vA_sb[:, gkb], pt2A[:, half],
                                         start=(kb == 0), stop=(kb == N_KB - 1))
                        nc.tensor.matmul(oB_ps[:], vB_sb[:, gkb], pt2B[:, half],
                                         start=(kb == 0), stop=(kb == N_KB - 1))
                    if pkp == N_KP - 1:
                        # iteration finished: one PSUM read per head (with
                        # the rowsum row), then pack the rowsums from SBUF.
                        oA_raw = pt_pool.tile([HD + 1, NF], F32, tag="oraw", bufs=6)
                        oB_raw = pt_pool.tile([HD + 1, NF], F32, tag="oraw", bufs=6)
                        nc.vector.tensor_copy(oA_raw[:], oA_ps[:])
                        nc.vector.tensor_copy(oB_raw[:], oB_ps[:])
                        del o_tiles[pit]
                        zp = pt_pool.tile([HD + 1, NF], F32, tag="zp", bufs=2)
                        nc.vector.tensor_copy(zp[0:1, :], oA_raw[HD : HD + 1, :])
                        nc.vector.tensor_copy(zp[HD : HD + 1, :], oB_raw[HD : HD + 1, :])
                        pending = (oA_raw, oB_raw, zp, pq0, pit)

                def sec_filler():
                    nonlocal y_done
                    if g >= total_pairs:
                        return
                    spent = 0
                    while filler and filler[0][0] <= g + 2:
                        ent = filler.pop(0)
                        emit_filler(ent)
                        spent += ent[2]
                    target = 1500 if filler else 1100
                    # the kp==1 slot must bridge the previous iteration's
                    # PSUM evictions (the o-ring reuse), so it may draw the
                    # y reserve down further
                    floor = 3 if kp == 1 else 6
                    ny = 0
                    while spent < target:
                        if filler:
                            ent = filler.pop(0)
                            emit_filler(ent)
                            spent += ent[2]
                        elif y_avail and ny < 2 and len(y_avail) > floor:
                            emit_ypair(y_avail.pop(0))
                            y_done += 1
                            ny += 1
                            spent += 1024
                        else:
                            break

                # At kp==1 the previous slot's PV closed an iteration and its
                # PSUM evictions are still draining on DVE; the o-ring reuse
                # in this slot's PV would stall the in-order PE, so let the
                # filler run between ST and PV there. Everywhere else the PV
                # goes first so boundary evictions beat filler epilogues into
                # the DVE queue.
                sec_st()
                if kp == 1:
                    sec_filler()
                    sec_pv()
                else:
                    sec_pv()
                    sec_filler()

            # ---- drain: last iteration's normalization + remaining y.
            # The reserved y tiles are emitted BEFORE the apply (matmul
            # stationary reads of oT_sb are dependency-tracked coarsely, so
            # anything after the apply waits for it) and evict via ACT,
            # which is idle after the recip — the DVE queue stays clear for
            # the apply itself.
            norm_state = emit_recip(*pending)
            bc_state = emit_bcast(*norm_state)
            rest = y_avail[:]
            y_avail.clear()
            for s0 in rest:
                emit_ypair(s0, drain=True)
            emit_apply(*bc_state)        # appends the last iteration's tiles
            for s0 in y_avail:
                emit_ypair(s0, drain=True)

    nc.compile()
    return nc


def kernel(x, Wq, bq, Wk, bk, Wv, bv, Wo, bo, _trace=False):
    global last_results
    x = np.asarray(x, dtype=np.float32)
    Wq, bq = np.asarray(Wq, np.float32), np.asarray(bq, np.float32)
    Wk, bk = np.asarray(Wk, np.float32), np.asarray(bk, np.float32)
    Wv, bv = np.asarray(Wv, np.float32), np.asarray(bv, np.float32)
    Wo, bo = np.asarray(Wo, np.float32), np.asarray(bo, np.float32)

    if "nc" not in _cache:
        _cache["nc"] = _build()
    nc = _cache["nc"]

    dt_qk, dt_v, dt_out = _np_dt(DT_QK), _np_dt(DT_V), _np_dt(DT_OUT)
    # xs[p, sb, a, s'] = x[sb*NF+s', a*P+p] — per-partition contiguous strips
    x2 = x.reshape(BS, D)
    xs = np.ascontiguousarray(
        x2.reshape(N_SB, NF, N_DC, P).transpose(3, 0, 2, 1).reshape(P, -1)
    ).astype(dt_qk, copy=False)

    def chunk_w(W, sl, dt):
        # w[p, a, e] = W[sl][e, a*P+p]
        u = np.ascontiguousarray(W[sl].T).reshape(N_DC, P, ES)
        return np.ascontiguousarray(u.transpose(1, 0, 2).reshape(P, -1)).astype(dt, copy=False)

    in_maps = []
    for c in range(NCORES):
        sl = slice(c * ES, (c + 1) * ES)
        in_maps.append({
            "xs": xs,
            "wqc": chunk_w(Wq, sl, dt_qk),
            "wkc": chunk_w(Wk, sl, dt_qk),
            "wvc": chunk_w(Wv, sl, dt_v),
            "bq": np.ascontiguousarray(bq[sl, None]),
            "bk": np.ascontiguousarray(bk[sl, None]),
            "bv": np.ascontiguousarray(bv[None, sl]),
            "woT": np.ascontiguousarray(Wo[:, sl].T).astype(dt_out, copy=False),
        })

    res = bass_utils.run_bass_kernel_spmd(
        nc, in_maps, core_ids=list(range(NCORES)), trace=_trace)
    last_results = res

    y = res.results[0]["y"].astype(np.float64)
    for c in range(1, NCORES):
        y += res.results[c]["y"]
    y = (y + bo).astype(np.float32)
    return y.reshape(B, S, D)
